# revision 69
# baseline (speedup 1.0000x reference)
"""GAT layer (nn_GATLayerAdj) Trainium2 Bass kernel, 8-core SPMD.

Reference computation (N=1024, di=do=64):
    a[i,j]  = x[j]@w_src + x[i]@w_tgt + bw        (attention logits)
    att     = softmax_j(where(adj>0, a, -1e16))
    y[i,j,:]= relu(x[j]@WfS.T + x[i]@WfT.T + bf)
    o[i,:]  = sum_j att[i,j] * y[i,j,:]

Key factorization: e[i,j] = exp(a[i,j])*M[i,j] with M = (adj>0) splits as
exp(atgt[i]+bw) * exp(asrc[j]) * M[i,j]; the row factor cancels in the
softmax, so att[i,j] = g[j]M[i,j] / sum_j g[j]M[i,j] with g = exp(asrc).
The device needs NO exp / softmax / transposes: the host uploads
e'^T[j,i] = g[j]*M[i,j] (transposed, PE-stationary-ready) and
r_t[i] = 1/sum_j e'^T[j,i] (same O(N^2) prep class as the old adjm
mask); all O(N^2 d) work runs on device.

Sharding: target-node dim i split across 8 cores (128 target rows each).

Per-core schedule (source dim j on partitions), QUARTER-PASS order:
pass q processes free columns [2048q, 2048q+2048) of all 8 chunks, so
u-broadcast slices are consumed strictly in arrival order and no
mid-kernel DMA wait occurs.
  1. u is replicated to all 128 partitions on the HOST so device DMAs
     are plain contiguous rows. DMAs ride three queues by need-time:
     sync HWDGE (head blob + u quarters 0-1 + outputs), act HWDGE
     (etp + rinv + u quarter 2), gpsimd SWDGE (u quarter 3).
  2. Per (chunk, quarter): z = ys_bcast + urep on DVE (tensor_tensor,
     2x bf16, [128,2048]); relu per a balance table on DVE
     (tensor_scalar_max, 4x) or ACT; then 4 reduce matmuls
     (b-group = q, 4x32 PSUM partitions via tile_position).
  3. A filler matmul (result discarded into a spare PSUM bank, operand
     = the freshly added z) after each quarter keeps the PE's HAM
     clock-gate warm: without it the PE idles >3us between matmul
     groups and drops to half clock for the rest of the kernel.
  4. After the final pass's chunk-7 matmul for bank n2, that bank
     evacuates (scale=1/s', DVE/ACT alternating) and streams out.

Numerics: bf16 inputs to the adds/matmuls, fp32 accumulation, bf16
output (host upcasts).
"""

from contextlib import ExitStack

import numpy as np
import ml_dtypes

import concourse.bass as bass
import concourse.tile as tile
from concourse import bacc, mybir
from concourse.bass_utils import run_bass_kernel_spmd

# Lighter TileContext exit: stock emits drain + full butterfly barrier +
# sem clears + second butterfly (~11us). Engines already sync at program
# end; keep the drain (output DMA completion), a sem-only rendezvous
# before the clears, and drop the trailing barrier.
import concourse.tile as _tile_mod

if not getattr(_tile_mod, "_exit_trimmed", False):
    def _drain_and_barrier_trim(self, tick_clock, wait_clock):
        from concourse.tile import ScopedClock
        nc = self.nc
        drain_inst = nc.sync.drain()
        wait_clock.add_sem_waits(
            drain_inst.ins, ScopedClock({None: tick_clock.global_clock})
        )
        exit_sem = nc.alloc_semaphore("exit_rdv")
        for eng in (nc.sync, nc.tensor, nc.vector, nc.scalar):
            eng.nop(nofuse=True).then_inc(exit_sem, 1)
        nc.gpsimd.wait_ge(exit_sem, 4)
        assert self.sems is not None
        popped = nc._tile_sem_poison_stack.pop()
        assert popped is self._sem_poison
        nc.clear_and_free_semaphores(list(self.sems.allocated().values()))
        nc.gpsimd.sem_clear(range(exit_sem.num, exit_sem.num + 1))

    _tile_mod.TileContext._drain_and_barrier = _drain_and_barrier_trim
    _tile_mod._exit_trimmed = True

N = 1024
DI = 64
DO = 64
N_CORES = 8
ROWS = N // N_CORES          # 128 target rows per core
NCHUNK = N // 128            # 8 j-chunks
F_FULL = ROWS * DO           # 8192 free size of (i, d)
QUART = F_FULL // 4          # 2048

f32 = mybir.dt.float32
bf16 = mybir.dt.bfloat16
AF = mybir.ActivationFunctionType
ALU = mybir.AluOpType

YW = NCHUNK * DO             # 512 ysjp cols
# head blob: [ysjp 512 | urep cols 0:512]
BLOB_W = DO + 512   # [ysjp chunk-0 cols | urep cols 0:512]

# Build form per (pass q, chunk c). The device holds nurep = -u:
#   'A': z = ys - nurep = ys+u on DVE (TT subtract, 2x), relu on ACT,
#        matmuls consume the relu'd tile.
#   'D': z = max(ys, nurep) = relu(ys+u) - u on DVE (one TT max, no
#        relu pass at all); since sum_j e'[j,i]*u[i,d] = s'[i]*u[i,d],
#        the host adds back (s'_D[i]/s'[i])*u[i,d] where s'_D sums e'
#        over the 'D' chunks of i's quarter.
# 12 D quarters; last quarter is D so the tail chain is short.
# Balance: DVE = 23x1.22 + 12x1.43 + evac ~= 42.5; ACT = 20x2.0 +
# 3 evac ~= 42.7.
RELU_ENG = [
    "ADAADAAD",
    "DAADDADA",
    "ADADADAD",
    "DAADADAD",
]

_CACHE = {}


def _build_program():
    nc = bacc.Bacc("TRN2", target_bir_lowering=False, debug=False,
                   num_devices=N_CORES)

    # ---- DRAM I/O ----
    blob_d = nc.dram_tensor("blob", [128, BLOB_W], bf16,
                            kind="ExternalInput").ap()
    u0_d = nc.dram_tensor("u0", [128, 512], bf16, kind="ExternalInput").ap()
    u1_d = nc.dram_tensor("u1", [128, 1536], bf16, kind="ExternalInput").ap()
    u2_d = nc.dram_tensor("u2", [128, 2048], bf16, kind="ExternalInput").ap()
    u3_d = nc.dram_tensor("u3", [128, 2048], bf16, kind="ExternalInput").ap()
    u4_d = nc.dram_tensor("u4", [128, 2048], bf16, kind="ExternalInput").ap()
    etp_d = nc.dram_tensor("etp", [128, N], bf16,
                           kind="ExternalInput").ap()
    rinv_d = nc.dram_tensor("rinv", [128, 1], f32, kind="ExternalInput").ap()
    ysr_d = nc.dram_tensor("ysr", [128, YW - DO], bf16,
                           kind="ExternalInput").ap()
    o_d = nc.dram_tensor("o", [128, 2048], bf16, kind="ExternalOutput").ap()

    with tile.TileContext(nc) as tc, ExitStack() as ctx:
        cons = ctx.enter_context(tc.tile_pool(name="cons", bufs=1))
        zp = ctx.enter_context(tc.tile_pool(name="zp", bufs=6))
        rp = ctx.enter_context(tc.tile_pool(name="rp", bufs=8))
        accp = ctx.enter_context(tc.tile_pool(name="accp", bufs=1, space="PSUM"))

        blob = cons.tile([128, BLOB_W], bf16)
        urep = cons.tile([128, F_FULL], bf16)
        etp = cons.tile([128, N], bf16)
        r_t = cons.tile([ROWS, 1], f32)
        ysr = cons.tile([128, YW - DO], bf16)

        # ---- DMAs on three queues, ordered by need-time. u cols 0:512
        # are uploaded twice (blob for chunk 0's first sub-adds, urep
        # for the rest) so every quarter AP stays within one tile. The
        # gpsimd SWDGE queue measures ~2.4x faster than the HWDGE
        # queues (~240 vs ~100 GB/s), so it carries the bulk urep.
        nc.sync.dma_start(blob[:], blob_d[:, :])
        nc.sync.dma_start(urep[:, 0:512], u0_d[:, :])
        nc.sync.dma_start(urep[:, 512:1024], u1_d[:, 0:512])
        nc.gpsimd.dma_start(urep[:, 1024:2048], u1_d[:, 512:1536])
        nc.gpsimd.dma_start(urep[:, 2048:4096], u2_d[:, :])
        nc.gpsimd.dma_start(urep[:, 4096:6144], u3_d[:, :])
        nc.gpsimd.dma_start(urep[:, 6144:8192], u4_d[:, :])
        nc.scalar.dma_start(ysr[:], ysr_d[:, :])
        nc.scalar.dma_start(etp[:], etp_d[:, :])
        nc.scalar.dma_start(r_t[:], rinv_d[:, :])

        et_all = etp[:, 0:N]

        def ys_c_ap(c):
            # chunk 0's ys rides the head blob; chunks 1-7 arrive on
            # the act queue
            if c == 0:
                return blob[:, 0:DO]
            return ysr[:, DO * (c - 1):DO * c]

        def usl(c0, c1, from_blob=False):
            # u columns [c0, c1): chunk 0's first sub-adds read the
            # early blob copy, everything else the full urep tile
            if from_blob and c1 <= 512:
                return blob[:, DO + c0:DO + c1]
            return urep[:, c0:c1]

        t_accs = [accp.tile([128, 512], f32, tag=f"acc{n2}", name=f"t_acc{n2}")
                  for n2 in range(4)]
        fill_b = accp.tile([128, 512], f32, tag="fill", name="fill_b")
        t_sb = cons.tile([128, 2048], bf16)

        # preload ACT's relu table during the DMA head so the first
        # real relu doesn't pay the ~1.3us ACT_TABLE_LOAD mid-kernel
        nc.scalar.activation(t_sb[0:1, 0:1], t_sb[0:1, 0:1], AF.Relu)

        def emit_add(c, z, q, parts, op, pos0=0):
            # z[:, zl] = ys_c (bcast over i) OP nurep[qcols], sub-steps
            ys_c = ys_c_ap(c)
            pos = pos0
            for step in parts:
                sl = (QUART * q + pos, QUART * q + pos + step)
                ys_b = ys_c.rearrange("p d -> p () d").broadcast_to(
                    (128, step // DO, DO))
                zv = z[:, pos:pos + step].rearrange(
                    "p (i d) -> p i d", i=step // DO)
                uv = usl(*sl, from_blob=(c == 0 and q == 0)).rearrange(
                    "p (i d) -> p i d", i=step // DO)
                nc.vector.tensor_tensor(zv, ys_b, uv, op)
                pos += step

        def emit_filler(z):
            # PE keep-warm filler: fires as soon as z (pre-relu) exists,
            # bridging the idle window while the relu runs (an idle gap
            # >3us drops the HAM clock gate to half rate).
            nc.tensor.matmul(fill_b[0:32, :], et_all[:, 0:32], z[:, 0:512],
                             start=True, stop=True, skip_group_check=True)

        def emit_mms(q, c, r, base, first, last, mms=range(4)):
            for n2 in mms:
                nc.tensor.matmul(
                    t_accs[n2][32 * q:32 * (q + 1), :],
                    et_all[:, 128 * c + 32 * q:128 * c + 32 * q + 32],
                    r[:, base + 512 * n2:base + 512 * (n2 + 1)],
                    start=first,
                    stop=last,
                    skip_group_check=True,
                    tile_position=(0, 32 * q),
                )
                if last and q == 3:
                    # bank n2 fully accumulated: scaled evacuation
                    # (mostly ACT, which is otherwise idle; bank 2 on
                    # DVE so the last two banks drain in parallel);
                    # bank pairs stream out as one 2KB-row DMA on the
                    # fast gpsimd queue
                    osl = slice(512 * n2, 512 * (n2 + 1))
                    if n2 == 2:
                        nc.vector.tensor_scalar_mul(t_sb[:, osl],
                                                    t_accs[n2][:, :], r_t[:])
                    else:
                        nc.scalar.activation(t_sb[:, osl], t_accs[n2][:, :],
                                             AF.Copy, bias=0.0, scale=r_t[:])
                    if n2 % 2 == 1:
                        psl = slice(512 * (n2 - 1), 512 * (n2 + 1))
                        nc.gpsimd.dma_start(out=o_d[:, psl],
                                            in_=t_sb[:, psl])

        def emit_quarter(q, c, first, last):
            eng = RELU_ENG[q][c]
            z = zp.tile([128, QUART], bf16, name="z")
            subs = (512, 512, 1024) if (q, c) == (0, 0) else (QUART,)
            emit_add(c, z, q, subs,
                     ALU.max if eng == "D" else ALU.subtract)
            emit_filler(z)
            if eng == "D":
                emit_mms(q, c, z, 0, first, last)
            else:
                r = rp.tile([128, QUART], bf16, name="r")
                nc.scalar.activation(r[:], z[:], AF.Relu)
                emit_mms(q, c, r, 0, first, last)

        for q in range(4):
            for c in range(NCHUNK):
                emit_quarter(q, c, first=(c == 0), last=(c == NCHUNK - 1))

    nc.compile()
    return nc


def _prep_inputs(x, adj, Wf, bf_, Ww, bw):
    b = ml_dtypes.bfloat16
    x64 = x.astype(np.float64)
    ys = (x64 @ Wf[:, :DI].astype(np.float64).T).astype(np.float32)   # [N, 64]
    u = (x64 @ Wf[:, DI:].astype(np.float64).T + bf_).astype(np.float32)
    asrc = (x64 @ Ww[0, :DI].astype(np.float64)).astype(np.float32)   # [N]
    g = np.exp(asrc.astype(np.float64)).astype(np.float32)            # [N]

    # ysjp[jl, 64c+d] = ys[128c+jl, d]
    ysjp = ys.reshape(NCHUNK, 128, DO).transpose(1, 0, 2).reshape(128, -1)
    # e'^T[j, i] = g[j] * (adj[i, j] > 0), chunk-packed:
    # etp[jl, 128c+il] = e'^T[128c+jl, il]
    mask_t = (adj > 0).T.astype(np.float32)          # [j, i]
    et_full = mask_t * g[:, None]                    # [j, i]
    sfull = et_full.sum(axis=0)                      # [i] row sums (denom)

    in_maps = []
    uadds = []
    for c in range(N_CORES):
        blk = slice(ROWS * c, ROWS * (c + 1))
        et = et_full[:, blk]                          # [1024, 128]
        etp = et.reshape(NCHUNK, 128, ROWS).transpose(1, 0, 2).reshape(128, -1)
        # relu-free formulation: device computes max(ys, -u); upload -u
        nuflat = (-u[blk].reshape(F_FULL)).astype(b)  # [8192]
        ubc = np.ascontiguousarray(
            np.broadcast_to(nuflat, (128, F_FULL)))   # host-side replicate
        blob = np.empty((128, BLOB_W), b)
        blob[:, 0:DO] = ysjp[:, 0:DO].astype(b)
        blob[:, DO:] = ubc[:, 0:512]
        m = dict(
            blob=blob,
            u0=np.ascontiguousarray(ubc[:, 0:512]),
            u1=np.ascontiguousarray(ubc[:, 512:2048]),
            u2=np.ascontiguousarray(ubc[:, 2048:4096]),
            u3=np.ascontiguousarray(ubc[:, 4096:6144]),
            u4=np.ascontiguousarray(ubc[:, 6144:8192]),
            etp=np.ascontiguousarray(etp).astype(b),
            rinv=np.ascontiguousarray(
                (1.0 / sfull[blk]).reshape(128, 1)).astype(np.float32),
            ysr=np.ascontiguousarray(ysjp[:, DO:]).astype(b),
        )
        # addback weights: for output row i (quarter qi = (i%128)//32),
        # the 'D'-form chunks contributed e'*(relu(..) - u); add back
        # u * sum_{c in D(qi)} cs[c,i] / s'[i]
        cs = et.reshape(NCHUNK, 128, ROWS).sum(axis=1)    # [c, il]
        w = np.zeros(ROWS, np.float64)
        for il in range(ROWS):
            qi = il // 32
            dset = [cc for cc in range(NCHUNK) if RELU_ENG[qi][cc] == "D"]
            w[il] = cs[dset, il].sum() / sfull[blk][il]
        uadds.append(w[:, None].astype(np.float32) * u[blk])
        in_maps.append(m)
    _CACHE["uadd"] = np.concatenate(uadds, axis=0)        # [N, DO]
    return in_maps


def get_program():
    if "nc" not in _CACHE:
        _CACHE["nc"] = _build_program()
    return _CACHE["nc"]


def unpack_output(res_list):
    p_idx = np.arange(128)
    col0 = (p_idx % 32) * DO
    cols = col0[:, None] + np.arange(DO)[None, :]
    out = np.empty((N, DO), np.float32)
    for c in range(N_CORES):
        t = res_list[c]["o"].astype(np.float32)      # [128, 2048]
        out[ROWS * c:ROWS * (c + 1)] = t[p_idx[:, None], cols]
    # add back the u term the 'D'-form (max) chunks left out
    return out + _CACHE["uadd"]


def kernel(x, adj, Wf, bf, Ww, bw):
    x = np.asarray(x, dtype=np.float32)
    adj = np.asarray(adj, dtype=np.int32)
    Wf = np.asarray(Wf, dtype=np.float32)
    bf_ = np.asarray(bf, dtype=np.float32)
    Ww = np.asarray(Ww, dtype=np.float32)
    bw = np.asarray(bw, dtype=np.float32)
    assert x.shape == (N, DI) and adj.shape == (N, N)

    nc = get_program()
    in_maps = _prep_inputs(x, adj, Wf, bf_, Ww, bw)
    res = run_bass_kernel_spmd(nc, in_maps, core_ids=list(range(N_CORES)))
    return unpack_output(res.results)
